# revision 29
# baseline (speedup 1.0000x reference)
"""GAT kernel for TRN2: host prep + Bass program builder + numpy model.

Sharding: nodes (and their in-edges) partitioned across cores by contiguous
shard; per dst-block-of-128 selector-matmul scatter; edge gathers of packed
table rows [h bf16 (256B) | al f32 (16B) | pad] = 512B via gpsimd dma_gather
with a lo/hi table split (int16 index limit); inter-layer AllGather of the
table; BN via AllReduce of per-core partial sums; pooling via transposed
graph-selector matmul; tiny FC + final AllReduce.
"""
from dataclasses import dataclass

import numpy as np

import concourse.bacc as bacc
import concourse.bass as bass
import concourse.mybir as mybir
import concourse.tile as tile
from concourse import library_config

F32 = mybir.dt.float32
BF = mybir.dt.bfloat16
I16 = mybir.dt.int16
I8 = mybir.dt.int8
AX = mybir.AluOpType
AF = mybir.ActivationFunctionType


@dataclass
class Cfg:
    ncores: int = 8
    n_real: int = 50000       # real nodes
    np_: int = 50176          # padded nodes (multiple of ncores*128)
    e_raw: int = 800000       # edges before self loops
    g: int = 500              # graphs
    gp: int = 512             # padded graphs (pool matmul free dim)
    f: int = 128              # features (in = out = 128)
    h: int = 4
    c: int = 32
    k: int = 6
    eps: float = 1e-5
    rowf: int = 128           # table row f32-slots (512B)
    neg_slope: float = 0.2
    single_packet: bool = False
    # filled by prep_edges:
    tlo_b: tuple = ()         # per-block lo tile counts (max over cores)
    thi_b: tuple = ()

    @property
    def shard(self):
        return self.np_ // self.ncores

    @property
    def nblk(self):
        return self.shard // 128

    @property
    def half(self):
        return self.np_ // 2

    @property
    def hshard(self):
        return self.shard // 2

    @property
    def tpb_b(self):
        return tuple(l + h for l, h in zip(self.tlo_b, self.thi_b))

    @property
    def tpb_max(self):
        return max(self.tpb_b)

    @property
    def tot_lo(self):
        return sum(self.tlo_b)

    @property
    def tot_hi(self):
        return sum(self.thi_b)

    def offs(self):
        lo, hi, out = 0, 0, []
        for b in range(self.nblk):
            out.append((lo, hi))
            lo += self.tlo_b[b]
            hi += self.thi_b[b]
        return out


def fold_attn(a, H, C):
    A = np.zeros((H * C, H), np.float32)
    for h in range(H):
        A[h * C:(h + 1) * C, h] = a[h]
    return A


def pack_idx16(idx):
    """int array [n] (n % 128 == 0) -> [128, n//16] int16 dma_gather layout."""
    n = len(idx)
    arr = np.zeros((16, n // 16), dtype=np.int16)
    arr[np.arange(n) % 16, np.arange(n) // 16] = idx
    return np.tile(arr, (8, 1))


def prep_edges(cfg: Cfg, edge_index):
    """Compact per-(core, block) edge streams: lo tiles then hi tiles.

    Returns streams[ci][b] = (src_lo, dst_lo, src_hi, dst_hi) padded to
    tlo_b/thi_b tiles; sets cfg.tlo_b/thi_b.
    """
    n, sh = cfg.n_real, cfg.shard
    hs = cfg.hshard
    src = np.concatenate([edge_index[0], np.arange(n)]).astype(np.int64)
    dst = np.concatenate([edge_index[1], np.arange(n)]).astype(np.int64)
    # lo table = concat over cores of first half-shards (AllGather of
    # ht_in[:hshard]); hi = second halves. Table-local index of node v:
    #   lo: (v // sh) * hs + (v % sh)            when v % sh < hs
    #   hi: (v // sh) * hs + (v % sh) - hs       otherwise
    buckets = [[None] * cfg.nblk for _ in range(cfg.ncores)]
    tlo_b = [1] * cfg.nblk
    thi_b = [1] * cfg.nblk
    for ci in range(cfg.ncores):
        m = (dst // sh) == ci
        s, d = src[m], dst[m] - ci * sh
        for b in range(cfg.nblk):
            mb = (d // 128) == b
            sb, db = s[mb], d[mb] % 128
            lo = (sb % sh) < hs
            lo_idx = (sb[lo] // sh) * hs + (sb[lo] % sh)
            hi_idx = (sb[~lo] // sh) * hs + (sb[~lo] % sh) - hs
            buckets[ci][b] = ((lo_idx, db[lo]), (hi_idx, db[~lo]))
            tlo_b[b] = max(tlo_b[b], -(-len(lo_idx) // 128))
            thi_b[b] = max(thi_b[b], -(-len(hi_idx) // 128))
    cfg.tlo_b, cfg.thi_b = tuple(tlo_b), tuple(thi_b)
    streams = [[None] * cfg.nblk for _ in range(cfg.ncores)]
    for ci in range(cfg.ncores):
        for b in range(cfg.nblk):
            (slo, dlo), (shi, dhi) = buckets[ci][b]
            sl = np.zeros(tlo_b[b] * 128, np.int64)
            dl = np.full(tlo_b[b] * 128, 999, np.int64)
            sl[: len(slo)] = slo
            dl[: len(dlo)] = dlo
            sh_ = np.zeros(thi_b[b] * 128, np.int64)
            dh_ = np.full(thi_b[b] * 128, 999, np.int64)
            sh_[: len(shi)] = shi
            dh_[: len(dhi)] = dhi
            streams[ci][b] = (sl, dl, sh_, dh_)
    return streams


def prep_inputs(cfg: Cfg, inputs):
    """Build per-core in_maps (list of dicts)."""
    H, C, F = cfg.h, cfg.c, cfg.f
    streams = prep_edges(cfg, inputs["edge_index"])
    TPB = cfg.tpb_max

    xpad = np.zeros((cfg.np_, F), np.float32)
    xpad[: cfg.n_real] = inputs["x"]

    W1e = np.concatenate(
        [inputs["W1"], inputs["W1"] @ fold_attn(inputs["a_src1"], H, C),
         inputs["W1"] @ fold_attn(inputs["a_dst1"], H, C)], axis=1)  # [F,136]
    W2e = np.concatenate(
        [inputs["W2"], inputs["W2"] @ fold_attn(inputs["a_src2"], H, C),
         inputs["W2"] @ fold_attn(inputs["a_dst2"], H, C)], axis=1)

    batch = np.asarray(inputs["batch"]).astype(np.int64)
    batch_pad = np.full(cfg.np_, 999, np.int64)
    batch_pad[: cfg.n_real] = batch
    cnt = np.bincount(batch, minlength=cfg.gp).astype(np.float32)
    rcnt = (1.0 / np.maximum(cnt, 1.0)).astype(np.float32)

    shared = dict(
        w1e=W1e.astype(np.float32), w2e=W2e.astype(np.float32),
        b1bc=np.tile(inputs["b1"][None, :], (128, 1)).astype(np.float32),
        b2bc=np.tile(inputs["b2"][None, :], (128, 1)).astype(np.float32),
        g1row=inputs["g1"][None, :].astype(np.float32),
        be1row=inputs["be1"][None, :].astype(np.float32),
        g2row=inputs["g2"][None, :].astype(np.float32),
        be2row=inputs["be2"][None, :].astype(np.float32),
        fcw=inputs["fcW"].astype(np.float32),
        fcbbc=np.tile(inputs["fcb"][:, None], (1, cfg.gp)).astype(np.float32),
        rcntbc=np.tile(rcnt[None, :], (cfg.k, 1)).astype(np.float32),
        ident=np.eye(128, dtype=np.float32),
        diota=np.tile(np.arange(128, dtype=np.float32)[None, :], (128, 1)).astype(
            mybir.dt.np(BF)),
        piota=np.arange(128, dtype=np.float32)[:, None].copy(),
        giota=np.tile(np.arange(cfg.gp, dtype=np.float32)[None, :], (128, 1)),
        onesrow=np.ones((1, 128), np.float32),
    )

    in_maps = []
    for ci in range(cfg.ncores):
        lo_all, hi_all = [], []
        dstsel = np.zeros((128, cfg.nblk, TPB), np.float32)
        drow = np.full((cfg.nblk, TPB * 128), -1, np.int8)
        for b in range(cfg.nblk):
            sl, dl, sh_, dh_ = streams[ci][b]
            lo_all.append(pack_idx16(sl))
            hi_all.append(pack_idx16(sh_))
            dcat = np.concatenate([dl, dh_])  # [tpb_b*128] slot-ordered
            tpb_b = cfg.tpb_b[b]
            dstsel[:, b, :tpb_b] = dcat.reshape(tpb_b, 128).T
            drow[b, : tpb_b * 128] = np.where(dcat > 127, -1, dcat).astype(np.int8)
        idx_lo = np.concatenate(lo_all, axis=1)  # [128, tot_lo*8]
        idx_hi = np.concatenate(hi_all, axis=1)
        sl_ = slice(ci * cfg.shard, (ci + 1) * cfg.shard)
        nm = np.zeros((128, cfg.nblk), np.float32)
        bc = np.zeros((128, cfg.nblk), np.float32)
        ids = np.arange(ci * cfg.shard, (ci + 1) * cfg.shard)
        nm[:] = (ids.reshape(cfg.nblk, 128).T < cfg.n_real)
        bc[:] = batch_pad[ids].reshape(cfg.nblk, 128).T.astype(np.float32)
        xs = xpad[sl_].reshape(cfg.nblk, 128, F).transpose(1, 0, 2)
        in_maps.append(dict(
            x_shard=np.ascontiguousarray(xs).reshape(128, cfg.nblk * F),
            idx_lo=idx_lo, idx_hi=idx_hi,
            dstsel=dstsel.astype(mybir.dt.np(BF)),
            drow=drow,
            node_mask=nm, batchcol=bc,
            **shared,
        ))
    return in_maps


# ---------------------------------------------------------------------------
# numpy model (for validation at any cfg)
# ---------------------------------------------------------------------------

def numpy_forward(cfg: Cfg, inputs):
    H, C, F = cfg.h, cfg.c, cfg.f
    streams = prep_edges(cfg, inputs["edge_index"])
    xpad = np.zeros((cfg.np_, F), np.float32)
    xpad[: cfg.n_real] = inputs["x"]
    bf = mybir.dt.np(BF)

    def layer(xp, W, asrc, adst, b):
        We = np.concatenate([W, W @ fold_attn(asrc, H, C), W @ fold_attn(adst, H, C)], 1)
        tab = (xp @ We).astype(np.float32)
        tab_h = tab[:, :F].astype(bf).astype(np.float32)
        out = np.zeros((cfg.np_, F), np.float32)
        hs, sh = cfg.hshard, cfg.shard
        for ci in range(cfg.ncores):
            for bi in range(cfg.nblk):
                base = ci * cfg.shard + bi * 128
                sl, dl, sh_, dh_ = streams[ci][bi]
                g_lo = (sl // hs) * sh + (sl % hs)
                g_hi = (sh_ // hs) * sh + hs + (sh_ % hs)
                s = np.maximum(np.concatenate([g_lo, g_hi]), 0)
                d = np.concatenate([dl, dh_])
                ar_blk = tab[base: base + 128, F + H: F + 2 * H]
                sel = (d[:, None] == np.arange(128)[None, :]).astype(np.float32)
                e = tab[s][:, F:F + H] + sel @ ar_blk
                e = np.where(e > 0, e, cfg.neg_slope * e).astype(np.float32)
                p = np.exp(e).astype(np.float32)
                w = tab_h[s] * np.repeat(p, C, 1)
                acc = sel.T @ np.concatenate([w, p], 1)
                ssum = np.maximum(np.repeat(acc[:, F:], C, 1), 1e-30)
                out[base:base + 128] = acc[:, :F] / ssum + b
        return out

    def bn_elu(hh, g, be):
        s, ss = hh[:cfg.n_real].sum(0), (hh[:cfg.n_real] ** 2).sum(0)
        mu = s / cfg.n_real
        var = ss / cfg.n_real - mu ** 2
        sc = g / np.sqrt(var + cfg.eps)
        sh = be - mu * sc
        y = hh * sc + sh
        return (np.where(y > 0, y, np.exp(np.minimum(y, 0)) - 1)).astype(np.float32)

    h1 = layer(xpad, inputs["W1"], inputs["a_src1"], inputs["a_dst1"], inputs["b1"])
    h1n = bn_elu(h1, inputs["g1"], inputs["be1"])
    h2 = layer(h1n, inputs["W2"], inputs["a_src2"], inputs["a_dst2"], inputs["b2"])
    h2n = bn_elu(h2, inputs["g2"], inputs["be2"])

    batch = np.asarray(inputs["batch"]).astype(np.int64)
    gsel = np.zeros((cfg.n_real, cfg.gp), np.float32)
    gsel[np.arange(cfg.n_real), batch] = 1.0
    pooled = h2n[:cfg.n_real].T @ gsel
    fc = inputs["fcW"].T.astype(np.float32) @ pooled
    cnt = np.bincount(batch, minlength=cfg.gp).astype(np.float32)
    fc = fc / np.maximum(cnt, 1.0)[None, :] + inputs["fcb"][:, None]
    return fc[:, :cfg.g].T  # [g, k]


# ---------------------------------------------------------------------------
# Bass program
# ---------------------------------------------------------------------------

def build_nc(cfg: Cfg):
    NB = cfg.nblk
    F, H, C, RF = cfg.f, cfg.h, cfg.c, cfg.rowf
    FH = F + H
    SH = cfg.shard
    GP = cfg.gp
    TPB = cfg.tpb_max
    TLO = max(cfg.tlo_b)
    THI = max(cfg.thi_b)
    ALO = 64  # f32-slot offset of al in a row (h bf16 in slots 0..63)
    OFFS = cfg.offs()

    nc = bacc.Bacc("TRN2", target_bir_lowering=False, debug=False,
                   num_devices=cfg.ncores, num_swdge_queues=4)

    def ext(name, shape, dtype=F32):
        return nc.dram_tensor(name, shape, dtype, kind="ExternalInput")

    x_shard = ext("x_shard", [128, NB * F])
    idx_lo = ext("idx_lo", [128, cfg.tot_lo * 8], I16)
    idx_hi = ext("idx_hi", [128, cfg.tot_hi * 8], I16)
    dstsel_d = ext("dstsel", [128, NB, TPB], BF)
    drow_d = ext("drow", [NB, TPB * 128], I8)
    node_mask = ext("node_mask", [128, NB])
    batchcol = ext("batchcol", [128, NB])
    w1e = ext("w1e", [F, F + 2 * H])
    w2e = ext("w2e", [F, F + 2 * H])
    b1bc = ext("b1bc", [128, F])
    b2bc = ext("b2bc", [128, F])
    g1row = ext("g1row", [1, F])
    be1row = ext("be1row", [1, F])
    g2row = ext("g2row", [1, F])
    be2row = ext("be2row", [1, F])
    fcw = ext("fcw", [F, cfg.k])
    fcbbc = ext("fcbbc", [cfg.k, GP])
    rcntbc = ext("rcntbc", [cfg.k, GP])
    ident_d = ext("ident", [128, 128])
    diota_d = ext("diota", [128, 128], BF)
    piota_d = ext("piota", [128, 1])
    giota_d = ext("giota", [128, GP])
    onesrow_d = ext("onesrow", [1, 128])

    out_d = nc.dram_tensor("out", [cfg.k, GP], F32, kind="ExternalOutput")

    rg = [list(range(cfg.ncores))]
    shared_as = "Shared" if cfg.ncores > 4 else "Local"

    with tile.TileContext(nc) as tc:
        with (
            tc.tile_pool(name="dram", bufs=1, space="DRAM") as dpool,
            tc.tile_pool(name="persist", bufs=1) as pp,
            tc.tile_pool(name="consts", bufs=1) as cp,
            tc.tile_pool(name="work", bufs=2) as wp_pool,
            tc.tile_pool(name="big", bufs=1) as bp_pool,
            tc.tile_pool(name="gath", bufs=6) as gp_pool,
            tc.tile_pool(name="psum", bufs=2, space="PSUM") as ps_pool,
            tc.tile_pool(name="psum1", bufs=1, space="PSUM") as ps1_pool,
        ):
            nc.gpsimd.load_library(library_config.mlp)

            # ---- persistent SBUF ----
            h_cur = pp.tile([128, NB, F], F32)          # shard activations
            ar_sb = pp.tile([128, NB, H], BF)
            dstsel_sb = pp.tile([128, NB, TPB], BF)
            mask_sb = pp.tile([128, NB], F32)
            bcol_sb = pp.tile([128, NB], F32)

            # ---- constants ----
            w1e_sb = cp.tile([128, F + 2 * H], F32)
            w2e_sb = cp.tile([128, F + 2 * H], F32)
            b1bc_sb = cp.tile([128, F], F32)
            b2bc_sb = cp.tile([128, F], F32)
            ident = cp.tile([128, 128], F32)
            diota = cp.tile([128, 128], BF)
            piota = cp.tile([128, 1], F32)
            giota = cp.tile([128, GP], F32)
            onesrow = cp.tile([1, 128], F32)
            g1_sb = cp.tile([1, F], F32)
            be1_sb = cp.tile([1, F], F32)
            g2_sb = cp.tile([1, F], F32)
            be2_sb = cp.tile([1, F], F32)
            fcw_sb = cp.tile([128, cfg.k], F32)
            fcbbc_sb = cp.tile([cfg.k, GP], F32)
            rcnt_sb = cp.tile([cfg.k, GP], F32)

            for sb, d in [(w1e_sb, w1e), (w2e_sb, w2e), (b1bc_sb, b1bc),
                          (b2bc_sb, b2bc), (ident, ident_d), (diota, diota_d),
                          (piota, piota_d), (giota, giota_d),
                          (onesrow, onesrow_d), (g1_sb, g1row), (be1_sb, be1row),
                          (g2_sb, g2row), (be2_sb, be2row), (fcw_sb, fcw),
                          (fcbbc_sb, fcbbc), (rcnt_sb, rcntbc),
                          (dstsel_sb, dstsel_d),
                          (mask_sb, node_mask), (bcol_sb, batchcol),
                          (h_cur, x_shard)]:
                nc.sync.dma_start(sb[:], d[:])

            # ---- DRAM internals ----
            ht_in = [dpool.tile([SH, RF], F32, name=f"ht{i}_in") for i in (1, 2)]
            ht_lo = [dpool.tile([cfg.half, RF], F32, addr_space=shared_as,
                                name=f"ht{i}_lo") for i in (1, 2)]
            ht_hi = [dpool.tile([cfg.half, RF], F32, addr_space=shared_as,
                                name=f"ht{i}_hi") for i in (1, 2)]
            bn_in = [dpool.tile([1, 2 * F], F32, name=f"bn{i}_in") for i in (1, 2)]
            bn_out = [dpool.tile([1, 2 * F], F32, addr_space=shared_as, name=f"bn{i}_out")
                      for i in (1, 2)]
            fc_in = dpool.tile([cfg.k, GP], F32)
            fc_out = dpool.tile([cfg.k, GP], F32, addr_space=shared_as)

            # ================= helper phases =================

            def dense_phase(li, wext_sb):
                """h_cur -> table rows (ht_in) + ar_sb; then split AllGather."""
                hs = cfg.hshard
                ag1_after = hs // 128  # rows [0, hs) ready after this block
                for b in range(NB):
                    tr_ps = ps_pool.tile([128, 128], F32, tag="psA")
                    nc.tensor.transpose(tr_ps[:], h_cur[:, b, :], ident[:])
                    xT = wp_pool.tile([128, 128], F32, tag="xT")
                    nc.vector.tensor_copy(xT[:], tr_ps[:])
                    dp_ps = ps_pool.tile([128, F + 2 * H], F32, tag="psB")
                    nc.tensor.matmul(dp_ps[:], xT[:], wext_sb[:], start=True, stop=True)
                    row = wp_pool.tile([128, RF], F32, tag="row")
                    nc.vector.tensor_copy(
                        row[:, :64].bitcast(BF), dp_ps[:, :F])
                    nc.vector.tensor_copy(row[:, ALO:ALO + H], dp_ps[:, F:FH])
                    nc.vector.tensor_copy(ar_sb[:, b, :], dp_ps[:, FH:FH + H])
                    nc.sync.dma_start(
                        ht_in[li][b * 128:(b + 1) * 128, :ALO + H],
                        row[:, :ALO + H])
                    if b == ag1_after:
                        nc.gpsimd.collective_compute(
                            "AllGather", AX.bypass, replica_groups=rg,
                            ins=[ht_in[li][:hs, :]], outs=[ht_lo[li][:]])
                nc.gpsimd.collective_compute(
                    "AllGather", AX.bypass, replica_groups=rg,
                    ins=[ht_in[li][hs:, :]], outs=[ht_hi[li][:]])

            def scatter_phase(li, bbc_sb):
                """edge phase: skewed per-block gathers + selector matmuls.

                lo-gathers run SKEW_HI blocks ahead of hi-gathers, which run
                SKEW_C blocks ahead of compute: the first hi-gather waits on
                the second (hi) AllGather, and gpsimd's queue is in-order, so
                without the skew it would stall the lo stream behind it.
                """
                glo_t = {}
                ghi_t = {}

                def gather_lo(b):
                    TL = cfg.tlo_b[b]
                    lo_off, _ = OFFS[b]
                    ilo_t = gp_pool.tile([128, TLO * 8], I16, tag="ilo")
                    nc.sync.dma_start(ilo_t[:, :TL * 8],
                                      idx_lo[:, lo_off * 8:(lo_off + TL) * 8])
                    glo_t[b] = gp_pool.tile([128, TLO, RF], F32, tag="glo",
                                            name=f"glo{b}")
                    nc.gpsimd.dma_gather(
                        out_ap=glo_t[b][:, :TL, :], in_ap=ht_lo[li][:],
                        idxs_ap=ilo_t[:, :TL * 8],
                        num_idxs=TL * 128, num_idxs_reg=TL * 128, elem_size=RF,
                        queue_num=(b % 2) * 2,
                        single_packet=cfg.single_packet)

                def gather_hi(b):
                    TH = cfg.thi_b[b]
                    _, hi_off = OFFS[b]
                    ihi_t = gp_pool.tile([128, THI * 8], I16, tag="ihi")
                    nc.sync.dma_start(ihi_t[:, :TH * 8],
                                      idx_hi[:, hi_off * 8:(hi_off + TH) * 8])
                    ghi_t[b] = gp_pool.tile([128, THI, RF], F32, tag="ghi",
                                            name=f"ghi{b}")
                    nc.gpsimd.dma_gather(
                        out_ap=ghi_t[b][:, :TH, :], in_ap=ht_hi[li][:],
                        idxs_ap=ihi_t[:, :TH * 8],
                        num_idxs=TH * 128, num_idxs_reg=TH * 128, elem_size=RF,
                        queue_num=(b % 2) * 2 + 1,
                        single_packet=cfg.single_packet)

                SKEW_HI, SKEW_C = 2, 4
                for step in range(NB + SKEW_C):
                    if step < NB:
                        gather_lo(step)
                    if SKEW_HI <= step < NB + SKEW_HI:
                        gather_hi(step - SKEW_HI)
                    if step < SKEW_C:
                        continue
                    b = step - SKEW_C
                    TL, TH = cfg.tlo_b[b], cfg.thi_b[b]
                    TPBb = TL + TH
                    glo = glo_t.pop(b)
                    ghi = ghi_t.pop(b)
                    # selectors
                    sel = wp_pool.tile([128, TPB, 128], BF, tag="sel")
                    nc.vector.tensor_tensor(
                        sel[:, :TPBb, :],
                        dstsel_sb[:, b, :TPBb].unsqueeze(2).broadcast_to(
                            [128, TPBb, 128]),
                        diota[:].unsqueeze(1).broadcast_to([128, TPBb, 128]),
                        AX.is_equal)
                    drow_sb = wp_pool.tile([128, TPB * 128], I8, tag="drow")
                    nc.sync.dma_start(
                        drow_sb[:, :TPBb * 128],
                        drow_d[b, : TPBb * 128].unsqueeze(0).broadcast_to(
                            [128, TPBb * 128]))
                    selT = wp_pool.tile([128, TPB * 128], BF, tag="selT")
                    nc.vector.tensor_scalar(selT[:, :TPBb * 128],
                                            drow_sb[:, :TPBb * 128],
                                            piota[:], None, AX.is_equal)
                    # ar expand per tile -> [128, TPBb, H] psum
                    arx_ps = ps_pool.tile([128, TPB, H], F32, tag="psA")
                    for t in range(TPBb):
                        nc.tensor.matmul(arx_ps[:, t, :],
                                         selT[:, t * 128:(t + 1) * 128],
                                         ar_sb[:, b, :], start=True, stop=True)
                    # e = al + ar (2 adds: lo/hi); lrelu(x) = max(x, 0.2x); exp
                    e_sb = wp_pool.tile([128, TPB, H], F32, tag="e")
                    nc.vector.tensor_tensor(e_sb[:, :TL, :],
                                            glo[:, :TL, ALO:ALO + H],
                                            arx_ps[:, :TL, :], AX.add)
                    nc.vector.tensor_tensor(e_sb[:, TL:TPBb, :],
                                            ghi[:, :TH, ALO:ALO + H],
                                            arx_ps[:, TL:TPBb, :], AX.add)
                    esc = wp_pool.tile([128, TPB, H], F32, tag="esc")
                    nc.vector.tensor_scalar(esc[:, :TPBb, :], e_sb[:, :TPBb, :],
                                            cfg.neg_slope, None, AX.mult)
                    nc.vector.tensor_tensor(e_sb[:, :TPBb, :], e_sb[:, :TPBb, :],
                                            esc[:, :TPBb, :], AX.max)
                    wpt = wp_pool.tile([128, TPB, FH], BF, tag="wpt")
                    nc.scalar.activation(wpt[:, :TPBb, F:FH],
                                         e_sb[:, :TPBb, :], AF.Exp)
                    # w = h * p_expand
                    nc.vector.tensor_tensor(
                        wpt[:, :TL, :F].rearrange("p t (h c) -> p t h c", c=C),
                        glo[:, :TL, :ALO].bitcast(BF).rearrange(
                            "p t (h c) -> p t h c", c=C),
                        wpt[:, :TL, F:FH].unsqueeze(3).broadcast_to(
                            [128, TL, H, C]),
                        AX.mult)
                    nc.vector.tensor_tensor(
                        wpt[:, TL:TPBb, :F].rearrange("p t (h c) -> p t h c", c=C),
                        ghi[:, :TH, :ALO].bitcast(BF).rearrange(
                            "p t (h c) -> p t h c", c=C),
                        wpt[:, TL:TPBb, F:FH].unsqueeze(3).broadcast_to(
                            [128, TH, H, C]),
                        AX.mult)
                    # scatter matmuls
                    acc_ps = ps_pool.tile([128, FH], F32, tag="psB")
                    for t in range(TPBb):
                        nc.tensor.matmul(acc_ps[:], sel[:, t, :], wpt[:, t, :],
                                         start=(t == 0),
                                         stop=(t == TPBb - 1))
                    # divide + bias -> h_cur
                    s_sb = wp_pool.tile([128, H], F32, tag="s")
                    nc.vector.tensor_scalar(s_sb[:], acc_ps[:, F:FH], 1e-30,
                                            None, AX.max)
                    r_sb = wp_pool.tile([128, H], F32, tag="r")
                    nc.vector.reciprocal(r_sb[:], s_sb[:])
                    nc.vector.tensor_tensor(
                        h_cur[:, b, :].rearrange("p (h c) -> p h c", c=C),
                        acc_ps[:, :F].rearrange("p (h c) -> p h c", c=C),
                        r_sb[:].unsqueeze(2).broadcast_to([128, H, C]),
                        AX.mult)
                    nc.vector.tensor_tensor(h_cur[:, b, :], h_cur[:, b, :],
                                            bbc_sb[:], AX.add)

            def bn_elu_phase(li, g_sb, be_sb):
                bn_ps_s = ps1_pool.tile([1, F], F32, tag="ps1s")
                bn_ps_q = ps1_pool.tile([1, F], F32, tag="ps1q")
                for b in range(NB):
                    sq = wp_pool.tile([128, F], F32, tag="bnsq")
                    nc.scalar.activation(sq[:], h_cur[:, b, :], AF.Square)
                    nc.tensor.matmul(bn_ps_s[:], mask_sb[:, b:b + 1],
                                     h_cur[:, b, :],
                                     start=(b == 0), stop=(b == NB - 1))
                    nc.tensor.matmul(bn_ps_q[:], mask_sb[:, b:b + 1], sq[:],
                                     start=(b == 0), stop=(b == NB - 1))
                bn_sb = wp_pool.tile([1, 2 * F], F32, tag="bnrow")
                nc.vector.tensor_copy(bn_sb[:, :F], bn_ps_s[:])
                nc.vector.tensor_copy(bn_sb[:, F:], bn_ps_q[:])
                nc.sync.dma_start(bn_in[li][:], bn_sb[:])
                nc.gpsimd.collective_compute(
                    "AllReduce", AX.add, replica_groups=rg,
                    ins=[bn_in[li][:]], outs=[bn_out[li][:]])
                st = wp_pool.tile([1, 2 * F], F32, tag="bnst")
                nc.sync.dma_start(st[:], bn_out[li][:])
                # mu = s/n ; var = ss/n - mu^2
                mu = wp_pool.tile([1, F], F32, tag="mu")
                nc.vector.tensor_scalar(mu[:], st[:, :F], 1.0 / cfg.n_real, None,
                                        AX.mult)
                var = wp_pool.tile([1, F], F32, tag="var")
                nc.vector.tensor_scalar(var[:], st[:, F:], 1.0 / cfg.n_real, None,
                                        AX.mult)
                mu2 = wp_pool.tile([1, F], F32, tag="mu2")
                nc.scalar.activation(mu2[:], mu[:], AF.Square)
                nc.vector.tensor_tensor(var[:], var[:], mu2[:], AX.subtract)
                # rstd = 1/sqrt(var+eps)
                nc.vector.tensor_scalar(var[:], var[:], cfg.eps, None, AX.add)
                sd = wp_pool.tile([1, F], F32, tag="sd")
                nc.scalar.activation(sd[:], var[:], AF.Sqrt)
                rstd = wp_pool.tile([1, F], F32, tag="rstd")
                nc.vector.reciprocal(rstd[:], sd[:])
                # scale = g*rstd ; shift = be - mu*scale
                ssrow = wp_pool.tile([1, 2 * F], F32, tag="ssrow")
                nc.vector.tensor_tensor(ssrow[:, :F], g_sb[:], rstd[:], AX.mult)
                musc = wp_pool.tile([1, F], F32, tag="musc")
                nc.vector.tensor_tensor(musc[:], mu[:], ssrow[:, :F], AX.mult)
                nc.vector.tensor_tensor(ssrow[:, F:], be_sb[:], musc[:], AX.subtract)
                # broadcast via K=1 matmul
                bc_ps = ps1_pool.tile([128, 2 * F], F32, tag="ps1")
                nc.tensor.matmul(bc_ps[:], onesrow[:], ssrow[:], start=True, stop=True)
                bc_sb = wp_pool.tile([128, 2 * F], F32, tag="bnbcsb")
                nc.vector.tensor_copy(bc_sb[:], bc_ps[:])
                # normalize + elu over the whole shard at once
                sc_b = bc_sb[:, :F].unsqueeze(1).broadcast_to([128, NB, F])
                sh_b = bc_sb[:, F:].unsqueeze(1).broadcast_to([128, NB, F])
                nc.vector.tensor_tensor(h_cur[:], h_cur[:], sc_b, AX.mult)
                nc.vector.tensor_tensor(h_cur[:], h_cur[:], sh_b, AX.add)
                neg = bp_pool.tile([128, NB, F], F32, tag="neg")
                nc.vector.tensor_scalar(neg[:], h_cur[:], 0.0, None, AX.min)
                nc.scalar.activation(neg[:], neg[:], AF.Exp)
                nc.vector.tensor_scalar(h_cur[:], h_cur[:], 0.0, None, AX.max)
                nc.vector.tensor_tensor(h_cur[:], h_cur[:], neg[:], AX.add)
                nc.vector.tensor_scalar(h_cur[:], h_cur[:], -1.0, None, AX.add)

            # ================= program =================
            dense_phase(0, w1e_sb)
            scatter_phase(0, b1bc_sb)
            bn_elu_phase(0, g1_sb, be1_sb)
            dense_phase(1, w2e_sb)
            scatter_phase(1, b2bc_sb)
            bn_elu_phase(1, g2_sb, be2_sb)
            # pooling
            pool_ps = ps1_pool.tile([128, GP], F32, tag="ps1")
            for b in range(NB):
                gsel = wp_pool.tile([128, GP], F32, tag="gsel")
                nc.vector.tensor_scalar(gsel[:], giota[:],
                                        bcol_sb[:, b].unsqueeze(1), None,
                                        AX.is_equal)
                nc.tensor.matmul(pool_ps[:], h_cur[:, b, :], gsel[:],
                                 start=(b == 0), stop=(b == NB - 1))
            pool_sb = wp_pool.tile([128, GP], F32, tag="poolsb")
            nc.vector.tensor_copy(pool_sb[:], pool_ps[:])
            fc_ps = ps1_pool.tile([cfg.k, GP], F32, tag="ps1")
            nc.tensor.matmul(fc_ps[:], fcw_sb[:], pool_sb[:], start=True, stop=True)
            fc_sb = wp_pool.tile([cfg.k, GP], F32, tag="fcsb")
            nc.vector.tensor_copy(fc_sb[:], fc_ps[:])
            nc.sync.dma_start(fc_in[:], fc_sb[:])
            nc.gpsimd.collective_compute("AllReduce", AX.add, replica_groups=rg,
                                         ins=[fc_in[:]], outs=[fc_out[:]])
            fin = wp_pool.tile([cfg.k, GP], F32, tag="fin")
            nc.sync.dma_start(fin[:], fc_out[:])
            nc.vector.tensor_tensor(fin[:], fin[:], rcnt_sb[:], AX.mult)
            nc.vector.tensor_tensor(fin[:], fin[:], fcbbc_sb[:], AX.add)
            nc.sync.dma_start(out_d[:], fin[:])

    nc.compile()
    return nc


# ---------------------------------------------------------------------------
# harness entry point: full inputs in, full output out
# ---------------------------------------------------------------------------

_NC_CACHE = {}


def kernel(**inputs):
    """Full-input GAT forward on 8 NeuronCores. Returns [500, 6] float32."""
    from concourse.bass_utils import run_bass_kernel_spmd

    cfg = Cfg()
    in_maps = prep_inputs(cfg, inputs)
    key = (cfg.tlo_b, cfg.thi_b, cfg.single_packet)
    if key not in _NC_CACHE:
        _NC_CACHE[key] = build_nc(cfg)
    nc = _NC_CACHE[key]
    res = run_bass_kernel_spmd(nc, in_maps, core_ids=list(range(cfg.ncores)))
    out = res.results[0]["out"]
    return np.ascontiguousarray(out[:, :cfg.g].T).astype(np.float32)


# revision 33
# speedup vs baseline: 1.0715x; 1.0715x over previous
"""GAT kernel for TRN2: host prep + Bass program builder + numpy model.

Sharding: nodes (and their in-edges) partitioned across cores by contiguous
shard; per dst-block-of-128 selector-matmul scatter; edge gathers of packed
table rows [h bf16 (256B) | al f32 (16B) | pad] = 512B via gpsimd dma_gather
with a lo/hi table split (int16 index limit); inter-layer AllGather of the
table; BN via AllReduce of per-core partial sums; pooling via transposed
graph-selector matmul; tiny FC + final AllReduce.
"""
from dataclasses import dataclass

import numpy as np

import concourse.bacc as bacc
import concourse.bass as bass
import concourse.mybir as mybir
import concourse.tile as tile
from concourse import library_config

F32 = mybir.dt.float32
BF = mybir.dt.bfloat16
I16 = mybir.dt.int16
I8 = mybir.dt.int8
AX = mybir.AluOpType
AF = mybir.ActivationFunctionType


@dataclass
class Cfg:
    ncores: int = 8
    n_real: int = 50000       # real nodes
    np_: int = 50176          # padded nodes (multiple of ncores*128)
    e_raw: int = 800000       # edges before self loops
    g: int = 500              # graphs
    gp: int = 512             # padded graphs (pool matmul free dim)
    f: int = 128              # features (in = out = 128)
    h: int = 4
    c: int = 32
    k: int = 6
    eps: float = 1e-5
    rowf: int = 128           # table row f32-slots (512B)
    neg_slope: float = 0.2
    single_packet: bool = False
    # filled by prep_edges:
    tlo_b: tuple = ()         # per-block lo tile counts (max over cores)
    thi_b: tuple = ()

    @property
    def shard(self):
        return self.np_ // self.ncores

    @property
    def nblk(self):
        return self.shard // 128

    @property
    def half(self):
        return self.np_ // 2

    @property
    def hshard(self):
        return self.shard // 2

    @property
    def tpb_b(self):
        return tuple(l + h for l, h in zip(self.tlo_b, self.thi_b))

    @property
    def tpb_max(self):
        return max(self.tpb_b)

    @property
    def tot_lo(self):
        return sum(self.tlo_b)

    @property
    def tot_hi(self):
        return sum(self.thi_b)

    def offs(self):
        lo, hi, out = 0, 0, []
        for b in range(self.nblk):
            out.append((lo, hi))
            lo += self.tlo_b[b]
            hi += self.thi_b[b]
        return out


def fold_attn(a, H, C):
    A = np.zeros((H * C, H), np.float32)
    for h in range(H):
        A[h * C:(h + 1) * C, h] = a[h]
    return A


def pack_idx16(idx):
    """int array [n] (n % 128 == 0) -> [128, n//16] int16 dma_gather layout."""
    n = len(idx)
    arr = np.zeros((16, n // 16), dtype=np.int16)
    arr[np.arange(n) % 16, np.arange(n) // 16] = idx
    return np.tile(arr, (8, 1))


def prep_edges(cfg: Cfg, edge_index):
    """Compact per-(core, block) edge streams: lo tiles then hi tiles.

    Returns streams[ci][b] = (src_lo, dst_lo, src_hi, dst_hi) padded to
    tlo_b/thi_b tiles; sets cfg.tlo_b/thi_b.
    """
    n, sh = cfg.n_real, cfg.shard
    hs = cfg.hshard
    src = np.concatenate([edge_index[0], np.arange(n)]).astype(np.int64)
    dst = np.concatenate([edge_index[1], np.arange(n)]).astype(np.int64)
    # lo table = concat over cores of first half-shards (AllGather of
    # ht_in[:hshard]); hi = second halves. Table-local index of node v:
    #   lo: (v // sh) * hs + (v % sh)            when v % sh < hs
    #   hi: (v // sh) * hs + (v % sh) - hs       otherwise
    buckets = [[None] * cfg.nblk for _ in range(cfg.ncores)]
    tlo_b = [1] * cfg.nblk
    thi_b = [1] * cfg.nblk
    for ci in range(cfg.ncores):
        m = (dst // sh) == ci
        s, d = src[m], dst[m] - ci * sh
        for b in range(cfg.nblk):
            mb = (d // 128) == b
            sb, db = s[mb], d[mb] % 128
            lo = (sb % sh) < hs
            lo_idx = (sb[lo] // sh) * hs + (sb[lo] % sh)
            hi_idx = (sb[~lo] // sh) * hs + (sb[~lo] % sh) - hs
            buckets[ci][b] = ((lo_idx, db[lo]), (hi_idx, db[~lo]))
            tlo_b[b] = max(tlo_b[b], -(-len(lo_idx) // 128))
            thi_b[b] = max(thi_b[b], -(-len(hi_idx) // 128))
    cfg.tlo_b, cfg.thi_b = tuple(tlo_b), tuple(thi_b)
    streams = [[None] * cfg.nblk for _ in range(cfg.ncores)]
    for ci in range(cfg.ncores):
        for b in range(cfg.nblk):
            (slo, dlo), (shi, dhi) = buckets[ci][b]
            sl = np.zeros(tlo_b[b] * 128, np.int64)
            dl = np.full(tlo_b[b] * 128, 999, np.int64)
            sl[: len(slo)] = slo
            dl[: len(dlo)] = dlo
            sh_ = np.zeros(thi_b[b] * 128, np.int64)
            dh_ = np.full(thi_b[b] * 128, 999, np.int64)
            sh_[: len(shi)] = shi
            dh_[: len(dhi)] = dhi
            streams[ci][b] = (sl, dl, sh_, dh_)
    return streams


def prep_inputs(cfg: Cfg, inputs):
    """Build per-core in_maps (list of dicts)."""
    H, C, F = cfg.h, cfg.c, cfg.f
    streams = prep_edges(cfg, inputs["edge_index"])
    TPB = cfg.tpb_max

    xpad = np.zeros((cfg.np_, F), np.float32)
    xpad[: cfg.n_real] = inputs["x"]

    W1e = np.concatenate(
        [inputs["W1"], inputs["W1"] @ fold_attn(inputs["a_src1"], H, C),
         inputs["W1"] @ fold_attn(inputs["a_dst1"], H, C)], axis=1)  # [F,136]
    W2e = np.concatenate(
        [inputs["W2"], inputs["W2"] @ fold_attn(inputs["a_src2"], H, C),
         inputs["W2"] @ fold_attn(inputs["a_dst2"], H, C)], axis=1)

    batch = np.asarray(inputs["batch"]).astype(np.int64)
    batch_pad = np.full(cfg.np_, 999, np.int64)
    batch_pad[: cfg.n_real] = batch
    cnt = np.bincount(batch, minlength=cfg.gp).astype(np.float32)
    rcnt = (1.0 / np.maximum(cnt, 1.0)).astype(np.float32)

    shared = dict(
        w1e=W1e.astype(mybir.dt.np(BF)), w2e=W2e.astype(mybir.dt.np(BF)),
        b1bc=np.tile(inputs["b1"][None, :], (128, 1)).astype(np.float32),
        b2bc=np.tile(inputs["b2"][None, :], (128, 1)).astype(np.float32),
        g1row=inputs["g1"][None, :].astype(np.float32),
        g1col=inputs["g1"][:, None].astype(np.float32).copy(),
        be1col=inputs["be1"][:, None].astype(np.float32).copy(),
        be1row=inputs["be1"][None, :].astype(np.float32),
        g2row=inputs["g2"][None, :].astype(np.float32),
        be2row=inputs["be2"][None, :].astype(np.float32),
        fcw=inputs["fcW"].astype(np.float32),
        fcbbc=np.tile(inputs["fcb"][:, None], (1, cfg.gp)).astype(np.float32),
        rcntbc=np.tile(rcnt[None, :], (cfg.k, 1)).astype(np.float32),
        ident=np.eye(128, dtype=np.float32),
        diota=np.tile(np.arange(128, dtype=np.float32)[None, :], (128, 1)).astype(
            mybir.dt.np(BF)),
        piota=np.arange(128, dtype=np.float32)[:, None].copy(),
        giota=np.tile(np.arange(cfg.gp, dtype=np.float32)[None, :], (128, 1)),
        onesrow=np.ones((1, 128), np.float32),
    )

    in_maps = []
    for ci in range(cfg.ncores):
        lo_all, hi_all = [], []
        dstsel = np.zeros((128, cfg.nblk, TPB), np.float32)
        drow = np.full((cfg.nblk, TPB * 128), -1, np.int8)
        for b in range(cfg.nblk):
            sl, dl, sh_, dh_ = streams[ci][b]
            lo_all.append(pack_idx16(sl))
            hi_all.append(pack_idx16(sh_))
            dcat = np.concatenate([dl, dh_])  # [tpb_b*128] slot-ordered
            tpb_b = cfg.tpb_b[b]
            dstsel[:, b, :tpb_b] = dcat.reshape(tpb_b, 128).T
            drow[b, : tpb_b * 128] = np.where(dcat > 127, -1, dcat).astype(np.int8)
        idx_lo = np.concatenate(lo_all, axis=1)  # [128, tot_lo*8]
        idx_hi = np.concatenate(hi_all, axis=1)
        sl_ = slice(ci * cfg.shard, (ci + 1) * cfg.shard)
        nm = np.zeros((128, cfg.nblk), np.float32)
        bc = np.zeros((128, cfg.nblk), np.float32)
        ids = np.arange(ci * cfg.shard, (ci + 1) * cfg.shard)
        nm[:] = (ids.reshape(cfg.nblk, 128).T < cfg.n_real)
        bc[:] = batch_pad[ids].reshape(cfg.nblk, 128).T.astype(np.float32)
        xs = xpad[sl_].reshape(cfg.nblk, 128, F).transpose(2, 0, 1)  # [F, NB, n]
        in_maps.append(dict(
            x_t=np.ascontiguousarray(xs).reshape(128, cfg.nblk * 128).astype(
                mybir.dt.np(BF)),
            idx_lo=idx_lo, idx_hi=idx_hi,
            dstsel=dstsel.astype(mybir.dt.np(BF)),
            drow=drow,
            node_mask=nm, batchcol=bc,
            **shared,
        ))
    return in_maps


# ---------------------------------------------------------------------------
# numpy model (for validation at any cfg)
# ---------------------------------------------------------------------------

def numpy_forward(cfg: Cfg, inputs):
    H, C, F = cfg.h, cfg.c, cfg.f
    streams = prep_edges(cfg, inputs["edge_index"])
    xpad = np.zeros((cfg.np_, F), np.float32)
    xpad[: cfg.n_real] = inputs["x"]
    bf = mybir.dt.np(BF)

    def layer(xp, W, asrc, adst, b):
        We = np.concatenate([W, W @ fold_attn(asrc, H, C), W @ fold_attn(adst, H, C)], 1)
        tab = (xp @ We).astype(np.float32)
        tab_h = tab[:, :F].astype(bf).astype(np.float32)
        out = np.zeros((cfg.np_, F), np.float32)
        hs, sh = cfg.hshard, cfg.shard
        for ci in range(cfg.ncores):
            for bi in range(cfg.nblk):
                base = ci * cfg.shard + bi * 128
                sl, dl, sh_, dh_ = streams[ci][bi]
                g_lo = (sl // hs) * sh + (sl % hs)
                g_hi = (sh_ // hs) * sh + hs + (sh_ % hs)
                s = np.maximum(np.concatenate([g_lo, g_hi]), 0)
                d = np.concatenate([dl, dh_])
                ar_blk = tab[base: base + 128, F + H: F + 2 * H]
                sel = (d[:, None] == np.arange(128)[None, :]).astype(np.float32)
                e = tab[s][:, F:F + H] + sel @ ar_blk
                e = np.where(e > 0, e, cfg.neg_slope * e).astype(np.float32)
                p = np.exp(e).astype(np.float32)
                w = tab_h[s] * np.repeat(p, C, 1)
                acc = sel.T @ np.concatenate([w, p], 1)
                ssum = np.maximum(np.repeat(acc[:, F:], C, 1), 1e-30)
                out[base:base + 128] = acc[:, :F] / ssum + b
        return out

    def bn_elu(hh, g, be):
        s, ss = hh[:cfg.n_real].sum(0), (hh[:cfg.n_real] ** 2).sum(0)
        mu = s / cfg.n_real
        var = ss / cfg.n_real - mu ** 2
        sc = g / np.sqrt(var + cfg.eps)
        sh = be - mu * sc
        y = hh * sc + sh
        return (np.where(y > 0, y, np.exp(np.minimum(y, 0)) - 1)).astype(np.float32)

    h1 = layer(xpad, inputs["W1"], inputs["a_src1"], inputs["a_dst1"], inputs["b1"])
    h1n = bn_elu(h1, inputs["g1"], inputs["be1"])
    h2 = layer(h1n, inputs["W2"], inputs["a_src2"], inputs["a_dst2"], inputs["b2"])
    h2n = bn_elu(h2, inputs["g2"], inputs["be2"])

    batch = np.asarray(inputs["batch"]).astype(np.int64)
    gsel = np.zeros((cfg.n_real, cfg.gp), np.float32)
    gsel[np.arange(cfg.n_real), batch] = 1.0
    pooled = h2n[:cfg.n_real].T @ gsel
    fc = inputs["fcW"].T.astype(np.float32) @ pooled
    cnt = np.bincount(batch, minlength=cfg.gp).astype(np.float32)
    fc = fc / np.maximum(cnt, 1.0)[None, :] + inputs["fcb"][:, None]
    return fc[:, :cfg.g].T  # [g, k]


# ---------------------------------------------------------------------------
# Bass program
# ---------------------------------------------------------------------------

def build_nc(cfg: Cfg):
    NB = cfg.nblk
    F, H, C, RF = cfg.f, cfg.h, cfg.c, cfg.rowf
    FH = F + H
    SH = cfg.shard
    GP = cfg.gp
    TPB = cfg.tpb_max
    TLO = max(cfg.tlo_b)
    THI = max(cfg.thi_b)
    ALO = 64  # f32-slot offset of al in a row (h bf16 in slots 0..63)
    OFFS = cfg.offs()

    nc = bacc.Bacc("TRN2", target_bir_lowering=False, debug=False,
                   num_devices=cfg.ncores, num_swdge_queues=4)

    def ext(name, shape, dtype=F32):
        return nc.dram_tensor(name, shape, dtype, kind="ExternalInput")

    x_t = ext("x_t", [128, NB * 128], BF)
    g1col_d = ext("g1col", [F, 1])
    be1col_d = ext("be1col", [F, 1])
    idx_lo = ext("idx_lo", [128, cfg.tot_lo * 8], I16)
    idx_hi = ext("idx_hi", [128, cfg.tot_hi * 8], I16)
    dstsel_d = ext("dstsel", [128, NB, TPB], BF)
    drow_d = ext("drow", [NB, TPB * 128], I8)
    node_mask = ext("node_mask", [128, NB])
    batchcol = ext("batchcol", [128, NB])
    w1e = ext("w1e", [F, F + 2 * H], BF)
    w2e = ext("w2e", [F, F + 2 * H], BF)
    b1bc = ext("b1bc", [128, F])
    b2bc = ext("b2bc", [128, F])
    g1row = ext("g1row", [1, F])
    be1row = ext("be1row", [1, F])
    g2row = ext("g2row", [1, F])
    be2row = ext("be2row", [1, F])
    fcw = ext("fcw", [F, cfg.k])
    fcbbc = ext("fcbbc", [cfg.k, GP])
    rcntbc = ext("rcntbc", [cfg.k, GP])
    ident_d = ext("ident", [128, 128])
    diota_d = ext("diota", [128, 128], BF)
    piota_d = ext("piota", [128, 1])
    giota_d = ext("giota", [128, GP])
    onesrow_d = ext("onesrow", [1, 128])

    out_d = nc.dram_tensor("out", [cfg.k, GP], F32, kind="ExternalOutput")

    rg = [list(range(cfg.ncores))]
    shared_as = "Shared" if cfg.ncores > 4 else "Local"

    with tile.TileContext(nc) as tc:
        with (
            tc.tile_pool(name="dram", bufs=1, space="DRAM") as dpool,
            tc.tile_pool(name="persist", bufs=1) as pp,
            tc.tile_pool(name="consts", bufs=1) as cp,
            tc.tile_pool(name="work", bufs=2) as wp_pool,
            tc.tile_pool(name="big", bufs=1) as bp_pool,
            tc.tile_pool(name="gath", bufs=6) as gp_pool,
            tc.tile_pool(name="psum", bufs=2, space="PSUM") as ps_pool,
            tc.tile_pool(name="psum1", bufs=1, space="PSUM") as ps1_pool,
        ):
            nc.gpsimd.load_library(library_config.mlp)

            # ---- persistent SBUF ----
            h_cur = pp.tile([128, NB, F], F32)          # shard activations
            xT_all = pp.tile([128, NB, 128], BF)        # f-major activations
            ar_sb = pp.tile([128, NB, H], BF)
            dstsel_sb = pp.tile([128, NB, TPB], BF)
            mask_sb = pp.tile([128, NB], F32)
            bcol_sb = pp.tile([128, NB], F32)

            # ---- constants ----
            w1e_sb = cp.tile([128, F + 2 * H], BF)
            w2e_sb = cp.tile([128, F + 2 * H], BF)
            b1bc_sb = cp.tile([128, F], F32)
            b2bc_sb = cp.tile([128, F], F32)
            ident = cp.tile([128, 128], F32)
            diota = cp.tile([128, 128], BF)
            piota = cp.tile([128, 1], F32)
            giota = cp.tile([128, GP], F32)
            onesrow = cp.tile([1, 128], F32)
            g1_sb = cp.tile([1, F], F32)
            g1c_sb = cp.tile([F, 1], F32)
            be1c_sb = cp.tile([F, 1], F32)
            be1_sb = cp.tile([1, F], F32)
            g2_sb = cp.tile([1, F], F32)
            be2_sb = cp.tile([1, F], F32)
            fcw_sb = cp.tile([128, cfg.k], F32)
            fcbbc_sb = cp.tile([cfg.k, GP], F32)
            rcnt_sb = cp.tile([cfg.k, GP], F32)

            for sb, d in [(w1e_sb, w1e), (w2e_sb, w2e), (b1bc_sb, b1bc),
                          (b2bc_sb, b2bc), (ident, ident_d), (diota, diota_d),
                          (piota, piota_d), (giota, giota_d),
                          (onesrow, onesrow_d), (g1_sb, g1row), (be1_sb, be1row),
                          (g2_sb, g2row), (be2_sb, be2row), (fcw_sb, fcw),
                          (fcbbc_sb, fcbbc), (rcnt_sb, rcntbc),
                          (dstsel_sb, dstsel_d),
                          (g1c_sb, g1col_d), (be1c_sb, be1col_d),
                          (mask_sb, node_mask), (bcol_sb, batchcol),
                          (xT_all, x_t)]:
                nc.sync.dma_start(sb[:], d[:])

            # ---- DRAM internals ----
            ht_in = [dpool.tile([SH, RF], F32, name=f"ht{i}_in") for i in (1, 2)]
            ht_lo = [dpool.tile([cfg.half, RF], F32, addr_space=shared_as,
                                name=f"ht{i}_lo") for i in (1, 2)]
            ht_hi = [dpool.tile([cfg.half, RF], F32, addr_space=shared_as,
                                name=f"ht{i}_hi") for i in (1, 2)]
            bn_in = [dpool.tile([1, 2 * F], F32, name=f"bn{i}_in") for i in (1, 2)]
            bn_out = [dpool.tile([1, 2 * F], F32, addr_space=shared_as, name=f"bn{i}_out")
                      for i in (1, 2)]
            fc_in = dpool.tile([cfg.k, GP], F32)
            fc_out = dpool.tile([cfg.k, GP], F32, addr_space=shared_as)

            # ================= helper phases =================

            def dense_from_xT(li, wext_sb, sc_col=None, sh_col=None):
                """xT tiles -> table rows (ht_in) + ar_sb; split AllGather.

                When sc_col/sh_col are given, applies bn scale/shift + ELU to
                each f-major tile first (per-partition scalars).
                """
                hs = cfg.hshard
                ag1_after = hs // 128  # rows [0, hs) ready after this block
                for b in range(NB):
                    if sc_col is None:
                        xn = xT_all[:, b, :]
                    else:
                        xn_t = wp_pool.tile([128, 128], BF, tag="xn")
                        nc.vector.tensor_scalar(xn_t[:], xT_all[:, b, :],
                                                sc_col[:], sh_col[:],
                                                AX.mult, AX.add)
                        ng = wp_pool.tile([128, 128], F32, tag="xng")
                        nc.vector.tensor_scalar(ng[:], xn_t[:], 0.0, None,
                                                AX.min)
                        nc.scalar.activation(ng[:], ng[:], AF.Exp)
                        nc.vector.tensor_scalar(xn_t[:], xn_t[:], 0.0, None,
                                                AX.max)
                        nc.vector.tensor_tensor(xn_t[:], xn_t[:], ng[:], AX.add)
                        nc.vector.tensor_scalar(xn_t[:], xn_t[:], -1.0, None,
                                                AX.add)
                        xn = xn_t[:]
                    dp_ps = ps_pool.tile([128, F + 2 * H], F32, tag="psB")
                    nc.tensor.matmul(dp_ps[:], xn, wext_sb[:], start=True, stop=True)
                    row = wp_pool.tile([128, RF], F32, tag="row")
                    nc.vector.tensor_copy(
                        row[:, :64].bitcast(BF), dp_ps[:, :F])
                    nc.vector.tensor_copy(row[:, ALO:ALO + H], dp_ps[:, F:FH])
                    nc.vector.tensor_copy(ar_sb[:, b, :], dp_ps[:, FH:FH + H])
                    nc.sync.dma_start(
                        ht_in[li][b * 128:(b + 1) * 128, :ALO + H],
                        row[:, :ALO + H])
                    if b == ag1_after:
                        nc.gpsimd.collective_compute(
                            "AllGather", AX.bypass, replica_groups=rg,
                            ins=[ht_in[li][:hs, :]], outs=[ht_lo[li][:]])
                nc.gpsimd.collective_compute(
                    "AllGather", AX.bypass, replica_groups=rg,
                    ins=[ht_in[li][hs:, :]], outs=[ht_hi[li][:]])

            def scatter_phase(li, bbc_sb, post_hook=None):
                """edge phase: skewed per-block gathers + selector matmuls.

                lo-gathers run SKEW_HI blocks ahead of hi-gathers, which run
                SKEW_C blocks ahead of compute: the first hi-gather waits on
                the second (hi) AllGather, and gpsimd's queue is in-order, so
                without the skew it would stall the lo stream behind it.
                """
                glo_t = {}
                ghi_t = {}

                def gather_lo(b):
                    TL = cfg.tlo_b[b]
                    lo_off, _ = OFFS[b]
                    ilo_t = gp_pool.tile([128, TLO * 8], I16, tag="ilo")
                    nc.sync.dma_start(ilo_t[:, :TL * 8],
                                      idx_lo[:, lo_off * 8:(lo_off + TL) * 8])
                    glo_t[b] = gp_pool.tile([128, TLO, RF], F32, tag="glo",
                                            name=f"glo{b}")
                    nc.gpsimd.dma_gather(
                        out_ap=glo_t[b][:, :TL, :], in_ap=ht_lo[li][:],
                        idxs_ap=ilo_t[:, :TL * 8],
                        num_idxs=TL * 128, num_idxs_reg=TL * 128, elem_size=RF,
                        queue_num=(b % 2) * 2,
                        single_packet=cfg.single_packet)

                def gather_hi(b):
                    TH = cfg.thi_b[b]
                    _, hi_off = OFFS[b]
                    ihi_t = gp_pool.tile([128, THI * 8], I16, tag="ihi")
                    nc.sync.dma_start(ihi_t[:, :TH * 8],
                                      idx_hi[:, hi_off * 8:(hi_off + TH) * 8])
                    ghi_t[b] = gp_pool.tile([128, THI, RF], F32, tag="ghi",
                                            name=f"ghi{b}")
                    nc.gpsimd.dma_gather(
                        out_ap=ghi_t[b][:, :TH, :], in_ap=ht_hi[li][:],
                        idxs_ap=ihi_t[:, :TH * 8],
                        num_idxs=TH * 128, num_idxs_reg=TH * 128, elem_size=RF,
                        queue_num=(b % 2) * 2 + 1,
                        single_packet=cfg.single_packet)

                SKEW_HI, SKEW_C, SKEW_P = 2, 4, 6
                for step in range(NB + SKEW_P):
                    if step < NB:
                        gather_lo(step)
                    if SKEW_HI <= step < NB + SKEW_HI:
                        gather_hi(step - SKEW_HI)
                    if post_hook is not None and SKEW_P <= step:
                        post_hook(step - SKEW_P)
                    if not (SKEW_C <= step < NB + SKEW_C):
                        continue
                    b = step - SKEW_C
                    TL, TH = cfg.tlo_b[b], cfg.thi_b[b]
                    TPBb = TL + TH
                    glo = glo_t.pop(b)
                    ghi = ghi_t.pop(b)
                    # selectors
                    sel = wp_pool.tile([128, TPB, 128], BF, tag="sel")
                    nc.vector.tensor_tensor(
                        sel[:, :TPBb, :],
                        dstsel_sb[:, b, :TPBb].unsqueeze(2).broadcast_to(
                            [128, TPBb, 128]),
                        diota[:].unsqueeze(1).broadcast_to([128, TPBb, 128]),
                        AX.is_equal)
                    drow_sb = wp_pool.tile([128, TPB * 128], I8, tag="drow")
                    nc.sync.dma_start(
                        drow_sb[:, :TPBb * 128],
                        drow_d[b, : TPBb * 128].unsqueeze(0).broadcast_to(
                            [128, TPBb * 128]))
                    selT = wp_pool.tile([128, TPB * 128], BF, tag="selT")
                    nc.vector.tensor_scalar(selT[:, :TPBb * 128],
                                            drow_sb[:, :TPBb * 128],
                                            piota[:], None, AX.is_equal)
                    # ar expand per tile -> [128, TPBb, H] psum
                    arx_ps = ps_pool.tile([128, TPB, H], F32, tag="psA")
                    for t in range(TPBb):
                        nc.tensor.matmul(arx_ps[:, t, :],
                                         selT[:, t * 128:(t + 1) * 128],
                                         ar_sb[:, b, :], start=True, stop=True)
                    # e = al + ar (2 adds: lo/hi); lrelu(x) = max(x, 0.2x); exp
                    e_sb = wp_pool.tile([128, TPB, H], F32, tag="e")
                    nc.vector.tensor_tensor(e_sb[:, :TL, :],
                                            glo[:, :TL, ALO:ALO + H],
                                            arx_ps[:, :TL, :], AX.add)
                    nc.vector.tensor_tensor(e_sb[:, TL:TPBb, :],
                                            ghi[:, :TH, ALO:ALO + H],
                                            arx_ps[:, TL:TPBb, :], AX.add)
                    esc = wp_pool.tile([128, TPB, H], F32, tag="esc")
                    nc.vector.tensor_scalar(esc[:, :TPBb, :], e_sb[:, :TPBb, :],
                                            cfg.neg_slope, None, AX.mult)
                    nc.vector.tensor_tensor(e_sb[:, :TPBb, :], e_sb[:, :TPBb, :],
                                            esc[:, :TPBb, :], AX.max)
                    wpt = wp_pool.tile([128, TPB, FH], BF, tag="wpt")
                    nc.scalar.activation(wpt[:, :TPBb, F:FH],
                                         e_sb[:, :TPBb, :], AF.Exp)
                    # w = h * p_expand
                    nc.vector.tensor_tensor(
                        wpt[:, :TL, :F].rearrange("p t (h c) -> p t h c", c=C),
                        glo[:, :TL, :ALO].bitcast(BF).rearrange(
                            "p t (h c) -> p t h c", c=C),
                        wpt[:, :TL, F:FH].unsqueeze(3).broadcast_to(
                            [128, TL, H, C]),
                        AX.mult)
                    nc.vector.tensor_tensor(
                        wpt[:, TL:TPBb, :F].rearrange("p t (h c) -> p t h c", c=C),
                        ghi[:, :TH, :ALO].bitcast(BF).rearrange(
                            "p t (h c) -> p t h c", c=C),
                        wpt[:, TL:TPBb, F:FH].unsqueeze(3).broadcast_to(
                            [128, TH, H, C]),
                        AX.mult)
                    # scatter matmuls
                    acc_ps = ps_pool.tile([128, FH], F32, tag="psB")
                    for t in range(TPBb):
                        nc.tensor.matmul(acc_ps[:], sel[:, t, :], wpt[:, t, :],
                                         start=(t == 0),
                                         stop=(t == TPBb - 1))
                    # divide + bias -> h_cur
                    s_sb = wp_pool.tile([128, H], F32, tag="s")
                    nc.vector.tensor_scalar(s_sb[:], acc_ps[:, F:FH], 1e-30,
                                            None, AX.max)
                    r_sb = wp_pool.tile([128, H], F32, tag="r")
                    nc.vector.reciprocal(r_sb[:], s_sb[:])
                    nc.vector.tensor_tensor(
                        h_cur[:, b, :].rearrange("p (h c) -> p h c", c=C),
                        acc_ps[:, :F].rearrange("p (h c) -> p h c", c=C),
                        r_sb[:].unsqueeze(2).broadcast_to([128, H, C]),
                        AX.mult)
                    nc.vector.tensor_tensor(h_cur[:, b, :], h_cur[:, b, :],
                                            bbc_sb[:], AX.add)

            def make_stats_hook(li, bn_ps_s, bn_ps_q, with_transpose):
                def hook(b):
                    sq = wp_pool.tile([128, F], F32, tag="bnsq")
                    nc.scalar.activation(sq[:], h_cur[:, b, :], AF.Square)
                    nc.tensor.matmul(bn_ps_s[:], mask_sb[:, b:b + 1],
                                     h_cur[:, b, :],
                                     start=(b == 0), stop=(b == NB - 1))
                    nc.tensor.matmul(bn_ps_q[:], mask_sb[:, b:b + 1], sq[:],
                                     start=(b == 0), stop=(b == NB - 1))
                    if with_transpose:
                        tr_ps = ps_pool.tile([128, 128], F32, tag="psA")
                        nc.tensor.transpose(tr_ps[:], h_cur[:, b, :], ident[:])
                        nc.vector.tensor_copy(xT_all[:, b, :], tr_ps[:])
                return hook

            def bn_reduce(li, bn_ps_s, bn_ps_q):
                """stats PSUM -> AllReduce -> st row [1, 2F] in SBUF."""
                bn_sb = wp_pool.tile([1, 2 * F], F32, tag="bnrow")
                nc.vector.tensor_copy(bn_sb[:, :F], bn_ps_s[:])
                nc.vector.tensor_copy(bn_sb[:, F:], bn_ps_q[:])
                nc.sync.dma_start(bn_in[li][:], bn_sb[:])
                nc.gpsimd.collective_compute(
                    "AllReduce", AX.add, replica_groups=rg,
                    ins=[bn_in[li][:]], outs=[bn_out[li][:]])

            def bn_cols(li):
                """AllReduced stats -> per-feature sc/sh columns [128, 1]."""
                stc = wp_pool.tile([128, 2], F32, tag="bnstc")
                nc.sync.dma_start(
                    stc[:], bn_out[li][:].rearrange("o (j f) -> (o f) j", j=2))
                mu = wp_pool.tile([128, 1], F32, tag="muc")
                nc.vector.tensor_scalar(mu[:], stc[:, :1], 1.0 / cfg.n_real,
                                        None, AX.mult)
                var = wp_pool.tile([128, 1], F32, tag="varc")
                nc.vector.tensor_scalar(var[:], stc[:, 1:], 1.0 / cfg.n_real,
                                        None, AX.mult)
                mu2 = wp_pool.tile([128, 1], F32, tag="mu2c")
                nc.scalar.activation(mu2[:], mu[:], AF.Square)
                nc.vector.tensor_tensor(var[:], var[:], mu2[:], AX.subtract)
                nc.vector.tensor_scalar(var[:], var[:], cfg.eps, None, AX.add)
                sd = wp_pool.tile([128, 1], F32, tag="sdc")
                nc.scalar.activation(sd[:], var[:], AF.Sqrt)
                rstd = wp_pool.tile([128, 1], F32, tag="rstdc")
                nc.vector.reciprocal(rstd[:], sd[:])
                sc = wp_pool.tile([128, 1], F32, tag="scc")
                nc.vector.tensor_tensor(sc[:], g1c_sb[:], rstd[:], AX.mult)
                sh = wp_pool.tile([128, 1], F32, tag="shc")
                nc.vector.tensor_tensor(sh[:], mu[:], sc[:], AX.mult)
                nc.vector.tensor_tensor(sh[:], be1c_sb[:], sh[:], AX.subtract)
                return sc, sh

            def bn_rows_apply(li, g_sb, be_sb):
                """AllReduced stats -> normalize + ELU h_cur in place."""
                st = wp_pool.tile([1, 2 * F], F32, tag="bnst")
                nc.sync.dma_start(st[:], bn_out[li][:])
                mu = wp_pool.tile([1, F], F32, tag="mu")
                nc.vector.tensor_scalar(mu[:], st[:, :F], 1.0 / cfg.n_real, None,
                                        AX.mult)
                var = wp_pool.tile([1, F], F32, tag="var")
                nc.vector.tensor_scalar(var[:], st[:, F:], 1.0 / cfg.n_real, None,
                                        AX.mult)
                mu2 = wp_pool.tile([1, F], F32, tag="mu2")
                nc.scalar.activation(mu2[:], mu[:], AF.Square)
                nc.vector.tensor_tensor(var[:], var[:], mu2[:], AX.subtract)
                nc.vector.tensor_scalar(var[:], var[:], cfg.eps, None, AX.add)
                sd = wp_pool.tile([1, F], F32, tag="sd")
                nc.scalar.activation(sd[:], var[:], AF.Sqrt)
                rstd = wp_pool.tile([1, F], F32, tag="rstd")
                nc.vector.reciprocal(rstd[:], sd[:])
                ssrow = wp_pool.tile([1, 2 * F], F32, tag="ssrow")
                nc.vector.tensor_tensor(ssrow[:, :F], g_sb[:], rstd[:], AX.mult)
                musc = wp_pool.tile([1, F], F32, tag="musc")
                nc.vector.tensor_tensor(musc[:], mu[:], ssrow[:, :F], AX.mult)
                nc.vector.tensor_tensor(ssrow[:, F:], be_sb[:], musc[:], AX.subtract)
                bc_ps = ps1_pool.tile([128, 2 * F], F32, tag="ps1")
                nc.tensor.matmul(bc_ps[:], onesrow[:], ssrow[:], start=True, stop=True)
                bc_sb = wp_pool.tile([128, 2 * F], F32, tag="bnbcsb")
                nc.vector.tensor_copy(bc_sb[:], bc_ps[:])
                sc_b = bc_sb[:, :F].unsqueeze(1).broadcast_to([128, NB, F])
                sh_b = bc_sb[:, F:].unsqueeze(1).broadcast_to([128, NB, F])
                nc.vector.tensor_tensor(h_cur[:], h_cur[:], sc_b, AX.mult)
                nc.vector.tensor_tensor(h_cur[:], h_cur[:], sh_b, AX.add)
                neg = bp_pool.tile([128, NB, F], F32, tag="neg")
                nc.vector.tensor_scalar(neg[:], h_cur[:], 0.0, None, AX.min)
                nc.scalar.activation(neg[:], neg[:], AF.Exp)
                nc.vector.tensor_scalar(h_cur[:], h_cur[:], 0.0, None, AX.max)
                nc.vector.tensor_tensor(h_cur[:], h_cur[:], neg[:], AX.add)
                nc.vector.tensor_scalar(h_cur[:], h_cur[:], -1.0, None, AX.add)

            # ================= program =================
            dense_from_xT(0, w1e_sb)
            bn1_s = ps1_pool.tile([1, F], F32, tag="ps1s")
            bn1_q = ps1_pool.tile([1, F], F32, tag="ps1q")
            scatter_phase(0, b1bc_sb,
                          make_stats_hook(0, bn1_s, bn1_q, with_transpose=True))
            bn_reduce(0, bn1_s, bn1_q)
            sc1, sh1 = bn_cols(0)
            dense_from_xT(1, w2e_sb, sc1, sh1)
            bn2_s = ps1_pool.tile([1, F], F32, tag="ps1s")
            bn2_q = ps1_pool.tile([1, F], F32, tag="ps1q")
            scatter_phase(1, b2bc_sb,
                          make_stats_hook(1, bn2_s, bn2_q, with_transpose=False))
            bn_reduce(1, bn2_s, bn2_q)
            bn_rows_apply(1, g2_sb, be2_sb)
            # pooling
            pool_ps = ps1_pool.tile([128, GP], F32, tag="ps1")
            for b in range(NB):
                gsel = wp_pool.tile([128, GP], F32, tag="gsel")
                nc.vector.tensor_scalar(gsel[:], giota[:],
                                        bcol_sb[:, b].unsqueeze(1), None,
                                        AX.is_equal)
                nc.tensor.matmul(pool_ps[:], h_cur[:, b, :], gsel[:],
                                 start=(b == 0), stop=(b == NB - 1))
            pool_sb = wp_pool.tile([128, GP], F32, tag="poolsb")
            nc.vector.tensor_copy(pool_sb[:], pool_ps[:])
            fc_ps = ps1_pool.tile([cfg.k, GP], F32, tag="ps1")
            nc.tensor.matmul(fc_ps[:], fcw_sb[:], pool_sb[:], start=True, stop=True)
            fc_sb = wp_pool.tile([cfg.k, GP], F32, tag="fcsb")
            nc.vector.tensor_copy(fc_sb[:], fc_ps[:])
            nc.sync.dma_start(fc_in[:], fc_sb[:])
            nc.gpsimd.collective_compute("AllReduce", AX.add, replica_groups=rg,
                                         ins=[fc_in[:]], outs=[fc_out[:]])
            fin = wp_pool.tile([cfg.k, GP], F32, tag="fin")
            nc.sync.dma_start(fin[:], fc_out[:])
            nc.vector.tensor_tensor(fin[:], fin[:], rcnt_sb[:], AX.mult)
            nc.vector.tensor_tensor(fin[:], fin[:], fcbbc_sb[:], AX.add)
            nc.sync.dma_start(out_d[:], fin[:])

    nc.compile()
    return nc


# ---------------------------------------------------------------------------
# harness entry point: full inputs in, full output out
# ---------------------------------------------------------------------------

_NC_CACHE = {}


def kernel(**inputs):
    """Full-input GAT forward on 8 NeuronCores. Returns [500, 6] float32."""
    from concourse.bass_utils import run_bass_kernel_spmd

    cfg = Cfg()
    in_maps = prep_inputs(cfg, inputs)
    key = (cfg.tlo_b, cfg.thi_b, cfg.single_packet)
    if key not in _NC_CACHE:
        _NC_CACHE[key] = build_nc(cfg)
    nc = _NC_CACHE[key]
    res = run_bass_kernel_spmd(nc, in_maps, core_ids=list(range(cfg.ncores)))
    out = res.results[0]["out"]
    return np.ascontiguousarray(out[:, :cfg.g].T).astype(np.float32)
